# revision 19
# baseline (speedup 1.0000x reference)
"""Embedding-lookup MF model kernel for Trainium2 (8 NeuronCores).

reference math (B = 16384, D = 64):
    u   = user_table[x[:, 0]]          # [B, D]
    v   = item_table[x[:, 1]]          # [B, D]
    out = sigmoid(sum(u * v, -1))      # [B]

Strategy: data-parallel across the batch. Each of the 8 cores handles 2048
batch rows. The two tables are concatenated host-side into one [U+I, D]
f32 table (ids are < 100000 for both columns, so only that prefix of the
1M-row user table is ever referenced).

The TRN2 indirect-DMA primitive consumes exactly ONE index per destination
partition and fills that partition's dest extent contiguously from
table[idx[p]] (verified on HW; multi-index offset APs crash the exec unit).
Each gather instruction moves 128 rows: dest [128, 64] f32, offsets
[128, 1]. 2048 u-rows + 2048 v-rows per core = 32 gather instructions at
~1.1 us of Q7 desc-gen each — the hard floor for scattered gathers on this
hardware (the batched dma_gather ucode measured slower end-to-end: ~7.7
ns/desc plus a ~15 us one-shot ucode library load).

Raw Block structure (explicit semaphores, no Tile scheduler), tuned to
shave the edges of the desc-gen chain:
  - idx tile columns are stored in GATHER-EMISSION order and loaded in two
    DMAs (first chunk's 10 columns first), so the chain starts as soon as
    the 5-KB head slice lands instead of the full 16-KB tile;
  - DMA-completion semaphores ride only each chunk's LAST gather (per-queue
    FIFO retirement makes earlier transfers implied), keeping the gpsimd
    queue free of extra waits;
  - the final 1-block chunk fuses multiply+reduce into one
    tensor_tensor_reduce (per-partition scalar accumulate), trimming one
    DVE hop off the exposed tail.

Layout per core (P=128 partitions, NBLK=16 blocks):
    batch row  b = n*128 + p   lives at  partition p, block n
    idx  SBUF tile [128, 32] int32, column k = indices of the k-th emitted
      gather (chunk-ordered: u blocks then v blocks per chunk; v entries
      pre-offset by u_rows)
    gather tile tg [128, 2048] f32: u rows at cols [0,1024), v at [1024,2048)
    out [128, 16] f32: out[p, n] = sigmoid result of batch row n*128+p

Tapered chunking [5,5,5,1]: desc-gen for all 32 gathers is serial on the
Q7, so only the LAST chunk's DMA-receipt + compute + store chain is exposed
at the tail. Keep the last chunk minimal.
"""

import os

# A previously crashed process can leave the NeuronCores wedged
# (NRT_EXEC_UNIT_UNRECOVERABLE on the next run); requesting a core reset at
# runtime init is harmless otherwise and self-heals that state.
os.environ.setdefault("NEURON_RT_RESET_CORES", "1")

import numpy as np

import concourse.bass as bass
import concourse.mybir as mybir
from concourse import bacc
from concourse.bass_utils import run_bass_kernel_spmd

N_CORES = 8
P = 128
D = 64
B = 16384
BPC = B // N_CORES  # 2048 batch rows per core
NBLK = BPC // P  # 16 column blocks of 128 batch rows
CHUNK_BLOCKS = [5, 5, 5, 1]
HEAD = 2 * CHUNK_BLOCKS[0]  # idx cols in the head slice (chunk 0)


def _gather_order():
    """Logical block ids (u: 0..15, v: 16..31) in gather-emission order."""
    order, b0 = [], 0
    for nb in CHUNK_BLOCKS:
        b1 = b0 + nb
        order += list(range(b0, b1)) + list(range(NBLK + b0, NBLK + b1))
        b0 = b1
    return order


_ORDER = _gather_order()
_programs: dict = {}


def _chunk_of(k: int) -> int:
    tot = 0
    for c, nb in enumerate(CHUNK_BLOCKS):
        tot += 2 * nb
        if k < tot:
            return c
    raise ValueError(k)


def _build(cat_rows: int):
    """Build the single-core program (run SPMD on 8 cores)."""
    nc = bacc.Bacc(
        "TRN2",
        target_bir_lowering=False,
        debug=False,
        detect_race_conditions=False,
    )
    f32, i32 = mybir.dt.float32, mybir.dt.int32
    idx = nc.dram_tensor("idx", [P, 2 * NBLK], i32, kind="ExternalInput")
    tbl = nc.dram_tensor("tbl", [cat_rows, D], f32, kind="ExternalInput")
    out = nc.dram_tensor("out", [P, NBLK], f32, kind="ExternalOutput")

    # cumulative gather count after each chunk
    cum, tot = [], 0
    for nb in CHUNK_BLOCKS:
        tot += 2 * nb
        cum.append(tot)

    with (
        nc.Block() as block,
        nc.sbuf_tensor("t_idx", [P, 2 * NBLK], i32) as t_idx,
        nc.sbuf_tensor("tg", [P, 2 * NBLK * D], f32) as tg,
        nc.sbuf_tensor("tw", [P, max(CHUNK_BLOCKS) * D], f32) as tw,
        nc.sbuf_tensor("t_res", [P, NBLK], f32) as t_res,
        nc.sbuf_tensor("t_bias", [P, 1], f32) as t_bias,
        nc.semaphore("s_i1") as s_i1,  # head idx slice landed (+16)
        nc.semaphore("s_i2") as s_i2,  # rest of idx tile landed (+16)
        nc.semaphore("s_g0") as s_g0,  # chunk 0 gather completions (+16 each)
        nc.semaphore("s_g1") as s_g1,  # chunk 1
        nc.semaphore("s_g2") as s_g2,  # chunk 2
        nc.semaphore("s_g3") as s_g3,  # chunk 3
        nc.semaphore("s_v") as s_v,  # per-chunk reduce done (+1)
        nc.semaphore("s_a") as s_a,  # per-chunk sigmoid done (+1)
        nc.semaphore("s_o") as s_o,  # output stores (+16 each)
    ):
        s_gc = [s_g0, s_g1, s_g2, s_g3]

        @block.gpsimd
        def _(gpsimd: bass.BassGpSimd):
            gpsimd.dma_start(t_idx[:, 0:HEAD], idx[:, 0:HEAD]).then_inc(s_i1, 16)
            gpsimd.dma_start(
                t_idx[:, HEAD : 2 * NBLK], idx[:, HEAD : 2 * NBLK]
            ).then_inc(s_i2, 16)
            # a single accumulator sem would be racy: the 16 DMA engines
            # bump +1 each independently, so a cumulative threshold can be
            # met by partial completions of later gathers. One sem per chunk
            # makes "16 * gathers-in-chunk" exact.
            for k, j in enumerate(_ORDER):
                if k == 0:
                    gpsimd.wait_ge(s_i1, 16)
                elif k == HEAD:
                    gpsimd.wait_ge(s_i2, 16)
                gpsimd.indirect_dma_start(
                    out=tg[:, j * D : (j + 1) * D],
                    out_offset=None,
                    in_=tbl[:],
                    in_offset=bass.IndirectOffsetOnAxis(
                        ap=t_idx[:, k : k + 1], axis=0
                    ),
                ).then_inc(s_gc[_chunk_of(k)], 16)

        @block.vector
        def _(vector: bass.BassVectorEngine):
            vector.memset(t_bias[:], 0.0)
            b0 = 0
            for c, nb in enumerate(CHUNK_BLOCKS):
                b1 = b0 + nb
                vector.wait_ge(s_gc[c], 16 * 2 * nb)
                u_ap = tg[:, b0 * D : b1 * D]
                v_ap = tg[:, (NBLK + b0) * D : (NBLK + b1) * D]
                w = tw[:, 0 : nb * D]
                vector.tensor_mul(out=w, in0=u_ap, in1=v_ap)
                vector.tensor_reduce(
                    out=t_res[:, b0:b1],
                    in_=w.rearrange("p (n d) -> p n d", d=D),
                    axis=mybir.AxisListType.X,
                    op=mybir.AluOpType.add,
                ).then_inc(s_v, 1)
                b0 = b1

        @block.scalar
        def _(scalar: bass.BassScalarEngine):
            b0 = 0
            for c, nb in enumerate(CHUNK_BLOCKS):
                b1 = b0 + nb
                scalar.wait_ge(s_v, c + 1)
                scalar.activation(
                    out=t_res[:, b0:b1],
                    in_=t_res[:, b0:b1],
                    func=mybir.ActivationFunctionType.Sigmoid,
                    bias=t_bias[:],
                ).then_inc(s_a, 1)
                b0 = b1

        @block.sync
        def _(sync: bass.BassEngine):
            b0 = 0
            for c, nb in enumerate(CHUNK_BLOCKS):
                b1 = b0 + nb
                # a 1-column store slice is a non-contiguous DMA (stride-16
                # partition rows); widen the last store to 4 columns — the
                # extra columns rewrite identical, already-final values
                s0 = b0 if b1 - b0 >= 4 else b1 - 4
                sync.wait_ge(s_a, c + 1)
                sync.dma_start(out[:, s0:b1], t_res[:, s0:b1]).then_inc(s_o, 16)
                b0 = b1
            sync.wait_ge(s_o, 16 * len(CHUNK_BLOCKS))

    nc.compile()
    return nc


def _get_program(cat_rows: int):
    if cat_rows not in _programs:
        _programs[cat_rows] = _build(cat_rows)
    return _programs[cat_rows]


def _prep_idx(xs: np.ndarray, u_rows: int) -> np.ndarray:
    """[BPC, 2] int32 -> [128, 32] idx tile, columns in gather order."""
    iu = xs[:, 0].reshape(NBLK, P).T  # [P, NBLK]
    iv = xs[:, 1].reshape(NBLK, P).T + u_rows
    logical = np.concatenate([iu, iv], axis=1)  # [P, 32] in block order
    return np.ascontiguousarray(logical[:, _ORDER], dtype=np.int32)


def _run(x, user_table, item_table, **run_kwargs):
    x = np.asarray(x)
    ut = np.asarray(user_table, dtype=np.float32)
    it = np.asarray(item_table, dtype=np.float32)
    assert x.shape == (B, 2), x.shape
    xi = x.astype(np.int32)
    # ids from the reference's randint fill are < 100000; upload only the
    # prefix of the user table that can actually be referenced.
    u_rows = min(ut.shape[0], max(100_000, int(xi[:, 0].max()) + 1))
    cat = np.ascontiguousarray(np.concatenate([ut[:u_rows], it], axis=0))
    nc = _get_program(cat.shape[0])
    in_maps = []
    for k in range(N_CORES):
        xs = xi[k * BPC : (k + 1) * BPC]
        in_maps.append({"idx": _prep_idx(xs, u_rows), "tbl": cat})
    res = run_bass_kernel_spmd(nc, in_maps, list(range(N_CORES)), **run_kwargs)
    out = np.empty(B, np.float32)
    for k in range(N_CORES):
        out[k * BPC : (k + 1) * BPC] = res.results[k]["out"].T.ravel()
    return out, res


def kernel(x, user_table, item_table):
    out, _ = _run(x, user_table, item_table)
    return out


# revision 20
# speedup vs baseline: 1.1607x; 1.1607x over previous
"""Embedding-lookup MF model kernel for Trainium2 (8 NeuronCores).

reference math (B = 16384, D = 64):
    u   = user_table[x[:, 0]]          # [B, D]
    v   = item_table[x[:, 1]]          # [B, D]
    out = sigmoid(sum(u * v, -1))      # [B]

Strategy: data-parallel across the batch. Each of the 8 cores handles 2048
batch rows. The two tables are concatenated host-side into one [U+I, D]
table (user ids produced by the reference's randint fill are < 100000, so
only that prefix of the 1M-row user table is ever referenced; we upload a
prefix sized to the actual max id).

The TRN2 indirect-DMA primitive consumes exactly ONE index per destination
partition and fills that partition's dest extent contiguously from
table[idx[p]] (verified on HW). So each gather instruction moves 128 rows:
dest [128, 64] slice, offsets [128, 1]. 2048 u-rows + 2048 v-rows per core
= 32 gather instructions, pipelined with the DVE mul + segmented-reduce and
ACT sigmoid per chunk.

Layout per core (P=128 partitions, NBLK=16 blocks):
    batch row  b = n*128 + p   lives at  partition p, block n
    idx  SBUF tile [128, 32] int32: col n       = u-id of block n
                                    col 16 + n  = (u_rows + v-id) of block n
    gather tile tg [128, 2048] f32: u rows at cols [0,1024), v at [1024,2048)
"""

import os

# A previously crashed process can leave the NeuronCores wedged
# (NRT_EXEC_UNIT_UNRECOVERABLE on the next run); requesting a core reset at
# runtime init is harmless otherwise and self-heals that state.
os.environ.setdefault("NEURON_RT_RESET_CORES", "1")

import numpy as np

import concourse.bass as bass
import concourse.mybir as mybir
import concourse.tile as tile
from concourse import bacc
from concourse.bass_utils import run_bass_kernel_spmd

N_CORES = 8
P = 128
D = 64
B = 16384
BPC = B // N_CORES  # 2048 batch rows per core
NBLK = BPC // P  # 16 column blocks of 128 batch rows
# Tapered chunking: desc-gen for all 32 gathers is serial on the Q7, so only
# the LAST chunk's DMA-receipt + mul/reduce/sigmoid/store chain is exposed at
# the tail. Keep the last chunk minimal.
CHUNK_BLOCKS = [5, 5, 5, 1]

_programs: dict = {}


def _build(cat_rows: int):
    """Build the single-core program (run SPMD on 8 cores)."""
    nc = bacc.Bacc(
        "TRN2",
        target_bir_lowering=False,
        debug=False,
        detect_race_conditions=False,
    )
    idx = nc.dram_tensor("idx", [P, 2 * NBLK], mybir.dt.int32, kind="ExternalInput")
    tbl = nc.dram_tensor("tbl", [cat_rows, D], mybir.dt.float32, kind="ExternalInput")
    out = nc.dram_tensor("out", [P, NBLK], mybir.dt.float32, kind="ExternalOutput")

    with tile.TileContext(nc) as tc:
        with (
            tc.tile_pool(name="io", bufs=1) as io_pool,
            tc.tile_pool(name="prod", bufs=2) as prod_pool,
        ):
            t_idx = io_pool.tile([P, 2 * NBLK], mybir.dt.int32)
            nc.sync.dma_start(out=t_idx[:], in_=idx[:])
            tg = io_pool.tile([P, 2 * NBLK * D], mybir.dt.float32)
            t_res = io_pool.tile([P, NBLK], mybir.dt.float32)
            # zero bias tile for the sigmoid activation: avoids the const-AP
            # DMA the framework would otherwise emit ahead of the idx load
            t_bias = io_pool.tile([P, 1], mybir.dt.float32)
            nc.vector.memset(t_bias[:], 0.0)
            b0 = 0
            for nb in CHUNK_BLOCKS:
                b1 = b0 + nb
                # gather this chunk's u blocks and v blocks, one row per
                # partition per instruction
                for j in list(range(b0, b1)) + list(range(NBLK + b0, NBLK + b1)):
                    nc.gpsimd.indirect_dma_start(
                        out=tg[:, j * D : (j + 1) * D],
                        out_offset=None,
                        in_=tbl[:],
                        in_offset=bass.IndirectOffsetOnAxis(
                            ap=t_idx[:, j : j + 1], axis=0
                        ),
                    )
                w = prod_pool.tile([P, nb * D], mybir.dt.float32, tag="w")
                nc.vector.tensor_mul(
                    out=w[:],
                    in0=tg[:, b0 * D : b1 * D],
                    in1=tg[:, (NBLK + b0) * D : (NBLK + b1) * D],
                )
                rs = t_res[:, b0:b1]
                nc.vector.reduce_sum(
                    out=rs,
                    in_=w[:].rearrange("p (n d) -> p n d", d=D),
                    axis=mybir.AxisListType.X,
                )
                nc.scalar.activation(
                    out=rs,
                    in_=rs,
                    func=mybir.ActivationFunctionType.Sigmoid,
                    bias=t_bias[:],
                )
                # store each chunk as soon as its sigmoid lands; only the last
                # (1-block) store sits on the critical tail
                nc.sync.dma_start(out=out[:, b0:b1], in_=t_res[:, b0:b1])
                b0 = b1
    nc.compile()
    return nc


def _get_program(cat_rows: int):
    if cat_rows not in _programs:
        _programs[cat_rows] = _build(cat_rows)
    return _programs[cat_rows]


def _prep_idx(xs: np.ndarray, u_rows: int) -> np.ndarray:
    """[BPC, 2] int32 -> [128, 32] idx tile (u cols then offset v cols)."""
    iu = xs[:, 0].reshape(NBLK, P).T  # [P, NBLK]
    iv = xs[:, 1].reshape(NBLK, P).T + u_rows
    return np.ascontiguousarray(np.concatenate([iu, iv], axis=1), dtype=np.int32)


def _run(x, user_table, item_table, **run_kwargs):
    x = np.asarray(x)
    ut = np.asarray(user_table, dtype=np.float32)
    it = np.asarray(item_table, dtype=np.float32)
    assert x.shape == (B, 2), x.shape
    xi = x.astype(np.int32)
    # user ids from the reference's randint fill are < 100000; upload only
    # the prefix of the user table that can actually be referenced.
    u_rows = min(ut.shape[0], max(100_000, int(xi[:, 0].max()) + 1))
    cat = np.ascontiguousarray(np.concatenate([ut[:u_rows], it], axis=0))
    nc = _get_program(cat.shape[0])
    in_maps = []
    for k in range(N_CORES):
        xs = xi[k * BPC : (k + 1) * BPC]
        in_maps.append({"idx": _prep_idx(xs, u_rows), "tbl": cat})
    res = run_bass_kernel_spmd(nc, in_maps, list(range(N_CORES)), **run_kwargs)
    out = np.empty(B, np.float32)
    for k in range(N_CORES):
        out[k * BPC : (k + 1) * BPC] = res.results[k]["out"].T.ravel()
    return out, res


def kernel(x, user_table, item_table):
    out, _ = _run(x, user_table, item_table)
    return out

